# revision 14
# baseline (speedup 1.0000x reference)
"""Masked attention on 8 TRN2 NeuronCores — pure data-parallel over batch.

Full inputs:  q,k,v (16,2048,128) f32, mask (16,2048,2048) bool.
Output:       (16,2048,128) f32.

Per core (2 batches). Scores are computed transposed S^T[k,q] = K·Q^T in bf16
on the PE so the AV contraction lands on the partition axis. The exp+mask
elementwise pass (8.4M elems/core) is split across engines by k-block pair
(SCHR pair-averaged Schraudolph on DVE/Pool, exact ACT exp + mask-mul on
DVE (E16) or Pool (EU8P)) exactly as before, but the schedule is a single
SOFTWARE-PIPELINED GLOBAL STREAM over all 64 (batch, q-tile, pair) steps:

  - Per step g: QK(g) matmuls, elementwise(g), then the AV matmuls of step
    g-PD. AV lags globally, so a pass's trailing AV blocks interleave with
    the NEXT pass's QK matmuls in the PE queue instead of serializing the
    pass boundary (the baseline lost ~3.1us/pass to this).
  - Mask DMAs are merged to one m8 + one m16 load per q-tile (the HWDGE
    descriptor-generator is a single ~625ns/DMA device shared by SP/ACT/DVE
    queues; at 94 DMAs it was a 59us serial resource, now ~48 DMAs).
  - All loads for both batches are issued up-front in deadline order on SP;
    the first q-tile's masks ride ACT/DVE queues so the head doesn't wait
    on SP's ~650ns/DMA issue rate. Stores stay on SP — they sit after all
    loads in SP's in-order queue, so their norm-waits can't block a load.
  - AV accumulates [q,129] per slot with a ones-column in va giving the
    softmax denominator; normalization is DVE reciprocal + scale, stored
    as [q, d] rows from SP's queue (host unpermutes).
"""

import numpy as np
import ml_dtypes

B, S, D = 16, 2048, 128
N_CORES = 8
BPC = B // N_CORES   # batches per core
P = 128              # partitions
QW = 512             # q-tile width
NQT = S // QW        # q-tiles per batch
KB = S // P          # k-blocks per batch
NPAIR = KB // 2      # k-block pairs per q-tile pass
PD = 6               # AV matmuls pipelined this many pairs behind elementwise
SPSUM_BUFS = 3       # scores double/triple buffering (2 banks each)
AV_BUFS = 2          # AV accumulator tiles (1 bank each)
ATT_BUFS = 10
ATTE_BUFS = 12    # att_e (exp outputs); separate pool from SCHR intermediates
SCHRI_BUFS = 8    # att1/att2 SCHR intermediates
WARM_N = 14
M8_BUFS = 5
M16_BUFS = 5

# Per-pair elementwise path (8 entries, k-block pairs 0..7):
#   SCHR: pair-averaged Schraudolph, DVE stt + DVE/Pool tail (m8 bias codes)
#   E16:  ACT exp + DVE tensor_mul (m16 0/1 int16 rows)
#   EU8P: ACT exp + Pool stt-mul (m8 uint8 rows)
# Order within a pass: shortest-latency paths (E16) at the pair positions
# with the earliest/latest AV deadlines; the long SCHR chains sit at {2,6}
# so ACT's six serialized exps never run more than 3 back-to-back (exp
# completion spacing 1.038us vs QK spacing 0.857us — longer runs make the
# s_ps WAR bite), and their AV deadline (pair index + PD) has slack.
PATHS = ["E16", "EU8P", "SCHR", "EU8P", "EU8P", "EU8P", "SCHR", "E16"]
M16_PAIRS = [i for i, p in enumerate(PATHS) if p == "E16"]
M8_PAIRS = [i for i, p in enumerate(PATHS) if p in ("SCHR", "EU8P")]
M16_POS = {pr: i for i, pr in enumerate(M16_PAIRS)}
M8_POS = {pr: i for i, pr in enumerate(M8_PAIRS)}
SCHR_BIAS_LIVE = 16256.0 - 134.0
SCHR_M_LIVE = 252
SCHR_M_DEAD = 24
SCHR_MSCALE = SCHR_BIAS_LIVE / SCHR_M_LIVE  # 63.976...: m*this = bias

_NC = None
LAST_RESULT = None   # BassKernelResults of the most recent run (for profiling)


def _build_nc(bpc=BPC, s=S):
    import concourse.bacc as bacc
    import concourse.tile as tile
    from concourse import mybir

    bf16 = mybir.dt.bfloat16
    f32 = mybir.dt.float32
    i16 = mybir.dt.int16
    u8 = mybir.dt.uint8

    # qT is host-pre-scaled by a = 128/(ln2*sqrt(D)), so PSUM scores are
    # already in Schraudolph units; exact exp just rescales by ln2/128
    escale = float(np.log(2.0) / P)
    invs2 = float(1.0 / np.sqrt(2.0))
    mscale = float(SCHR_MSCALE)

    n16 = len(M16_PAIRS) * 2     # k-blocks in m16
    n8 = len(M8_PAIRS) * 2       # k-blocks in m8

    nc = bacc.Bacc()
    qT = nc.declare_dram_parameter("qT", [bpc, P, s], bf16, isOutput=False)
    kT = nc.declare_dram_parameter("kT", [bpc, P, s], bf16, isOutput=False)
    # va host-packed as [p, kb*(D+1)]: row p holds v[kb*128+p, :]+[1] per kb
    va = nc.declare_dram_parameter(
        "va", [bpc, P, KB * (D + 1)], bf16, isOutput=False
    )
    # m16[b, i*2P+p, q] int16, i indexes M16_PAIRS: 0/1 not-mask rows for
    # E16 pairs
    m16 = nc.declare_dram_parameter("m16", [bpc, n16 * P, s], i16, isOutput=False)
    # m8: uint8 rows for SCHR pairs (bias codes {252 live, 24 dead}) and
    # EU8P pairs ({1, 0} multiplier), indexed by M8_PAIRS position
    m8 = nc.declare_dram_parameter("m8", [bpc, n8 * P, s], u8, isOutput=False)
    # out packed [b, qt, p, sl*D+d] so each partition row is one contiguous
    # 1KB chunk (avoids the <512B DMA latency penalty); host unpermutes
    out = nc.declare_dram_parameter(
        "out", [bpc, NQT, P, 4 * D], bf16, isOutput=True
    )

    KCW = [2 * P, 2 * P, 4 * P, 4 * P, 4 * P]
    KCO = [0, 2 * P, 4 * P, 8 * P, 12 * P]

    with tile.TileContext(nc) as tc:
        with (
            tc.tile_pool(name="qk", bufs=2) as qkp,
            tc.tile_pool(name="vp", bufs=2) as vp,
            tc.tile_pool(name="m16p", bufs=M16_BUFS) as m16p,
            tc.tile_pool(name="m8p", bufs=M8_BUFS) as m8p,
            tc.tile_pool(name="attn", bufs=ATT_BUFS) as attnp,
            tc.tile_pool(name="attne", bufs=ATTE_BUFS) as attnep,
            tc.tile_pool(name="schri", bufs=SCHRI_BUFS) as schrip,
            tc.tile_pool(name="outp", bufs=3) as outp,
            tc.tile_pool(name="const", bufs=1) as constp,
            tc.tile_pool(name="rp", bufs=12) as rp,
            tc.tile_pool(name="spsum", bufs=SPSUM_BUFS, space="PSUM") as spsum,
            tc.tile_pool(name="avpsum", bufs=AV_BUFS, space="PSUM") as avpsum,
        ):
            # ACT warm-up: load the Exp table (~1.3us) during initial DMAs
            warm = constp.tile([P, 128], bf16, name="warm", tag="warm")
            nc.vector.memset(warm[:], 0.0)
            nc.scalar.activation(
                warm[:, 0:1], warm[:, 0:1], mybir.ActivationFunctionType.Exp
            )
            # PE warm-up burst: p-state ramps to full clock with sustained
            # busy; burn the initial DMA-wait window on dummy matmuls
            wp = avpsum.tile([P, 2, D + 1], f32, name="warm_mm", tag="av")
            for _ in range(WARM_N):
                nc.tensor.matmul(
                    wp[:, 0, 0:P], lhsT=warm[:], rhs=warm[:],
                    start=True, stop=True,
                )

            # ---- per-batch tiles + loads, deadline-ordered -------------
            kt_ch = [[None] * len(KCW) for _ in range(bpc)]
            qt_ch = [[None] * NQT for _ in range(bpc)]
            va_s = [None] * bpc
            m16_t = [[None] * NQT for _ in range(bpc)]
            m8_t = [[None] * NQT for _ in range(bpc)]

            for b in range(bpc):
                kt_ch[b] = [
                    qkp.tile([P, w], bf16, name="kt_ch", tag=f"kt{i}", bufs=2)
                    for i, w in enumerate(KCW)
                ]
                qt_ch[b] = [
                    qkp.tile([P, QW], bf16, name="qt_ch", tag="qt",
                             bufs=NQT + 2)
                    for _ in range(NQT)
                ]
                va_s[b] = vp.tile([P, KB, D + 1], bf16, name="va_s", tag="va")
                m16_t[b] = [
                    m16p.tile([P, n16, QW], i16, name="m16_t", tag="m16",
                              bufs=M16_BUFS)
                    for _ in range(NQT)
                ]
                m8_t[b] = [
                    m8p.tile([P, n8, QW], u8, name="m8_t", tag="m8",
                             bufs=M8_BUFS)
                    for _ in range(NQT)
                ]

            def _ldk(b, i, eng):
                eng.dma_start(
                    out=kt_ch[b][i][:], in_=kT[b, :, KCO[i]:KCO[i] + KCW[i]]
                )

            def _ldq(b, qt, eng):
                eng.dma_start(
                    out=qt_ch[b][qt][:], in_=qT[b, :, qt * QW:(qt + 1) * QW]
                )

            def _ldva(b, lo, hi, eng):
                vr = va[b, :, :].rearrange("p (kb d) -> p kb d", d=D + 1)
                eng.dma_start(out=va_s[b][:, lo:hi, :], in_=vr[:, lo:hi, :])

            def _ldm16(b, qt, eng):
                m16r = m16[b, :, :].rearrange("(kb p) q -> p kb q", p=P)
                eng.dma_start(
                    out=m16_t[b][qt][:],
                    in_=m16r[:, :, qt * QW:(qt + 1) * QW],
                )

            def _ldm8(b, qt, eng, lo=0, hi=None):
                hi = n8 if hi is None else hi
                m8r = m8[b, :, :].rearrange("(kb p) q -> p kb q", p=P)
                eng.dma_start(
                    out=m8_t[b][qt][:, lo:hi, :],
                    in_=m8r[:, lo:hi, qt * QW:(qt + 1) * QW],
                )

            SP, ACT, POOL = nc.sync, nc.scalar, nc.gpsimd
            # batch 0, q-tile 0: masks split across ACT/Pool queues so the
            # first pass isn't gated on SP's ~650ns/DMA issue rate
            _ldk(0, 0, SP)
            _ldq(0, 0, SP)
            _ldm16(0, 0, SP)             # E16 pairs 0,7 (pair 0 first)
            _ldm8(0, 0, ACT, 0, 4)       # pairs 1,2
            _ldm8(0, 0, ACT, 4, 8)       # pairs 3,4
            _ldm8(0, 0, ACT, 8, 12)     # pairs 5,6
            _ldva(0, 0, 8, POOL)
            _ldva(0, 8, KB, POOL)
            _ldk(0, 1, SP)
            _ldk(0, 2, SP)
            _ldk(0, 3, SP)
            _ldk(0, 4, SP)
            for qt in range(1, NQT):
                _ldq(0, qt, SP)
                _ldm8(0, qt, SP)
                _ldm16(0, qt, SP)
            # batch 1: all on SP, deadline order
            _ldk(1, 0, SP)
            _ldq(1, 0, SP)
            _ldva(1, 0, 8, SP)
            _ldm8(1, 0, SP)
            _ldm16(1, 0, SP)
            _ldk(1, 1, SP)
            _ldva(1, 8, KB, SP)
            _ldk(1, 2, SP)
            _ldk(1, 3, SP)
            _ldk(1, 4, SP)
            for qt in range(1, NQT):
                _ldq(1, qt, SP)
                _ldm8(1, qt, SP)
                _ldm16(1, qt, SP)

            # ---- merged global pair stream -----------------------------
            def emit_norm(av_t, o_s, b_, qt_):
                for sl in range(4):
                    avap = av_t[sl // 2][:, sl % 2, :]
                    recip = rp.tile([P, 1], f32, name="recip", tag="recip")
                    nc.vector.reciprocal(recip[:], avap[:, D:D + 1])
                    nc.vector.tensor_scalar_mul(
                        o_s[:, sl, :], avap[:, 0:D], recip[:]
                    )
                nc.sync.dma_start(
                    out=out[b_, qt_, :, :].rearrange(
                        "p (sl d) -> p sl d", d=D
                    ),
                    in_=o_s[:, :, :],
                )

            TOT = bpc * NQT * NPAIR
            atts = [None] * TOT
            avs = {}    # pass index -> (av_t list, o_s)

            for g in range(TOT + PD):
                # AV block of step g-PD first: if QK(g) must wait on its
                # s_ps buffer (exp of g-SPSUM_BUFS still draining), the PE
                # does these matmuls instead of idling.
                gp = g - PD
                if gp >= 0:
                    bp = gp // (NQT * NPAIR)
                    tp = gp % NPAIR
                    pidx = gp // NPAIR
                    if tp == 0:
                        avs[pidx] = (
                            [
                                avpsum.tile(
                                    [P, 2, D + 1], f32, name="av", tag="av"
                                )
                                for _ in range(2)
                            ],
                            outp.tile([P, 4, D], bf16, name="o_s", tag="o_s"),
                        )
                    av_t, o_s = avs[pidx]
                    for h in range(2):
                        kb = 2 * tp + h
                        for sl in range(4):
                            nc.tensor.matmul(
                                av_t[sl // 2][:, sl % 2, :],
                                lhsT=atts[gp][:, h, sl * P:(sl + 1) * P],
                                rhs=va_s[bp][:, kb, :],
                                start=(kb == 0 and sl % 2 == 0),
                                stop=(kb == KB - 1 and sl % 2 == 1),
                            )
                    if tp == NPAIR - 1:
                        qtp = (gp // NPAIR) % NQT
                        emit_norm(av_t, o_s, bp, qtp)
                        del avs[pidx]

                if g < TOT:
                    b = g // (NQT * NPAIR)
                    qt = (g // NPAIR) % NQT
                    t = g % NPAIR
                    s_ps = spsum.tile([P, 2, QW], f32, name="s_ps", tag="s")
                    for h in range(2):
                        kb = 2 * t + h
                        kc = next(
                            i for i in range(len(KCW))
                            if KCO[i] <= kb * P < KCO[i] + KCW[i]
                        )
                        ko = kb * P - KCO[kc]
                        nc.tensor.matmul(
                            s_ps[:, h, :],
                            lhsT=kt_ch[b][kc][:, ko:ko + P],
                            rhs=qt_ch[b][qt][:],
                            start=True,
                            stop=True,
                        )
                    path = PATHS[t]
                    att = attnp.tile([P, 2, QW], bf16, name="att", tag="att")
                    if path == "SCHR":
                        pos = M8_POS[t]
                        att1 = schrip.tile(
                            [P, 2, QW], bf16, name="att1", tag="schri"
                        )
                        att2 = schrip.tile(
                            [P, 2, QW], bf16, name="att2", tag="schri"
                        )
                        # PSUM-reading ops stay on DVE (GPSIMD may not
                        # access PSUM); SBUF-only tail split DVE/Pool
                        nc.vector.scalar_tensor_tensor(
                            out=att1[:].bitcast(i16),
                            in0=m8_t[b][qt][:, 2 * pos:2 * pos + 2, :],
                            scalar=mscale,
                            in1=s_ps[:],
                            op0=mybir.AluOpType.mult,
                            op1=mybir.AluOpType.add,
                        )
                        nc.vector.tensor_scalar(
                            out=att2[:].bitcast(i16),
                            in0=att1[:].bitcast(i16),
                            scalar1=64,
                            scalar2=None,
                            op0=mybir.AluOpType.add,
                        )
                        nc.vector.tensor_scalar(
                            out=att2[:],
                            in0=att2[:],
                            scalar1=invs2,
                            scalar2=None,
                            op0=mybir.AluOpType.mult,
                        )
                        nc.gpsimd.tensor_add(att[:], att2[:], att1[:])
                    else:
                        att_e = attnep.tile(
                            [P, 2, QW], bf16, name="att_e", tag="atte"
                        )
                        nc.scalar.activation(
                            att_e[:],
                            s_ps[:],
                            mybir.ActivationFunctionType.Exp,
                            scale=escale,
                        )
                        if path == "E16":
                            pos = M16_POS[t]
                            nc.vector.tensor_mul(
                                att[:], att_e[:],
                                m16_t[b][qt][:, 2 * pos:2 * pos + 2, :],
                            )
                        else:  # EU8P
                            pos = M8_POS[t]
                            nc.gpsimd.tensor_mul(
                                att[:], att_e[:],
                                m8_t[b][qt][:, 2 * pos:2 * pos + 2, :],
                            )
                    atts[g] = att
    nc.compile()
    return nc


def _prep_inputs(q, k, v, mask):
    bf = ml_dtypes.bfloat16
    ones = np.ones((B, S, 1), dtype=np.float32)
    va_full = (
        np.concatenate([np.asarray(v, np.float32), ones], axis=2)
        .reshape(B, S // P, P, D + 1)
        .transpose(0, 2, 1, 3)
        .reshape(B, P, (S // P) * (D + 1))
        .astype(bf)
    )
    ascale = np.float32(P / np.log(2.0) / np.sqrt(D))
    qT_full = np.ascontiguousarray(
        np.asarray(q, np.float32).transpose(0, 2, 1) * ascale
    ).astype(bf)
    kT_full = np.ascontiguousarray(
        np.asarray(k, np.float32).transpose(0, 2, 1)
    ).astype(bf)
    mT = np.asarray(mask, bool).transpose(0, 2, 1)  # [b, k, q]
    m16_full = np.empty((B, len(M16_PAIRS) * 2 * P, S), dtype=np.int16)
    for i, pr in enumerate(M16_PAIRS):
        blk = mT[:, 2 * pr * P:(2 * pr + 2) * P, :]
        r0, r1 = 2 * i * P, (2 * i + 2) * P
        m16_full[:, r0:r1, :] = np.where(blk, np.int16(0), np.int16(1))
    m8_full = np.empty((B, len(M8_PAIRS) * 2 * P, S), dtype=np.uint8)
    for i, pr in enumerate(M8_PAIRS):
        blk = mT[:, 2 * pr * P:(2 * pr + 2) * P, :]
        r0, r1 = 2 * i * P, (2 * i + 2) * P
        if PATHS[pr] == "SCHR":
            m8_full[:, r0:r1, :] = np.where(
                blk, np.uint8(SCHR_M_DEAD), np.uint8(SCHR_M_LIVE)
            )
        else:
            m8_full[:, r0:r1, :] = (~blk).astype(np.uint8)
    return qT_full, kT_full, va_full, m16_full, m8_full


def kernel(q, k, v, mask, _trace=False, _trace_kwargs=None):
    global _NC, LAST_RESULT
    from concourse.bass_utils import run_bass_kernel_spmd

    if _NC is None:
        _NC = _build_nc()

    qT_full, kT_full, va_full, m16_full, m8_full = _prep_inputs(q, k, v, mask)

    in_maps = []
    for c in range(N_CORES):
        lo, hi = c * BPC, (c + 1) * BPC
        in_maps.append(
            {
                "qT": qT_full[lo:hi],
                "kT": kT_full[lo:hi],
                "va": va_full[lo:hi],
                "m16": m16_full[lo:hi],
                "m8": m8_full[lo:hi],
            }
        )

    kw = {}
    if _trace:
        kw["trace"] = True
        if _trace_kwargs:
            kw.update(_trace_kwargs)
    LAST_RESULT = run_bass_kernel_spmd(_NC, in_maps, list(range(N_CORES)), **kw)
    res = LAST_RESULT.results
    outp = np.concatenate(
        [np.asarray(res[c]["out"]) for c in range(N_CORES)], axis=0
    ).astype(np.float32)
    # [b, qt, p, sl*D+d] -> [b, qt*512 + sl*128 + p, d]
    out = (
        outp.reshape(B, NQT, P, 4, D)
        .transpose(0, 1, 3, 2, 4)
        .reshape(B, S, D)
    )
    return np.ascontiguousarray(out)


# revision 19
# speedup vs baseline: 1.0534x; 1.0534x over previous
"""Masked attention on 8 TRN2 NeuronCores — pure data-parallel over batch.

Full inputs:  q,k,v (16,2048,128) f32, mask (16,2048,2048) bool.
Output:       (16,2048,128) f32.

Per core (2 batches). Scores are computed transposed S^T[k,q] = K·Q^T in bf16
on the PE so the AV contraction lands on the partition axis. The exp+mask
elementwise pass (8.4M elems/core) is split across engines by k-block pair
(SCHR pair-averaged Schraudolph on DVE/Pool, exact ACT exp + mask-mul on
DVE (E16) or Pool (EU8P)) exactly as before, but the schedule is a single
SOFTWARE-PIPELINED GLOBAL STREAM over all 64 (batch, q-tile, pair) steps:

  - Per step g: QK(g) matmuls, elementwise(g), then the AV matmuls of step
    g-PD. AV lags globally, so a pass's trailing AV blocks interleave with
    the NEXT pass's QK matmuls in the PE queue instead of serializing the
    pass boundary (the baseline lost ~3.1us/pass to this).
  - Mask DMAs are merged to one m8 + one m16 load per q-tile (the HWDGE
    descriptor-generator is a single ~625ns/DMA device shared by SP/ACT/DVE
    queues; at 94 DMAs it was a 59us serial resource, now ~48 DMAs).
  - All loads for both batches are issued up-front in deadline order on SP;
    the first q-tile's masks ride ACT/DVE queues so the head doesn't wait
    on SP's ~650ns/DMA issue rate. Stores stay on SP — they sit after all
    loads in SP's in-order queue, so their norm-waits can't block a load.
  - AV accumulates [q,129] per slot with a ones-column in va giving the
    softmax denominator; normalization is DVE reciprocal + scale, stored
    as [q, d] rows from SP's queue (host unpermutes).
"""

import numpy as np
import ml_dtypes

B, S, D = 16, 2048, 128
N_CORES = 8
BPC = B // N_CORES   # batches per core
P = 128              # partitions
QW = 512             # q-tile width
NQT = S // QW        # q-tiles per batch
KB = S // P          # k-blocks per batch
NPAIR = KB // 2      # k-block pairs per q-tile pass
PD = 6               # AV matmuls pipelined this many pairs behind elementwise
SPSUM_BUFS = 3       # scores double/triple buffering (2 banks each)
AV_BUFS = 2          # AV accumulator tiles (1 bank each)
ATT_BUFS = 10
ATTE_BUFS = 12    # att_e (exp outputs); separate pool from SCHR intermediates
SCHRI_BUFS = 8    # att1/att2 SCHR intermediates
WARM_N = 14
M8_BUFS = 5
M16_BUFS = 5

# Per-pair elementwise path (8 entries, k-block pairs 0..7):
#   SCHR: pair-averaged Schraudolph, DVE stt + DVE/Pool tail (m8 bias codes)
#   E16:  ACT exp + DVE tensor_mul (m16 0/1 int16 rows)
#   EU8P: ACT exp + Pool stt-mul (m8 uint8 rows)
# Order within a pass: shortest-latency paths (E16) at the pair positions
# with the earliest/latest AV deadlines; the long SCHR chains sit at {2,6}
# so ACT's six serialized exps never run more than 3 back-to-back (exp
# completion spacing 1.038us vs QK spacing 0.857us — longer runs make the
# s_ps WAR bite), and their AV deadline (pair index + PD) has slack.
PATHS = ["E16", "EU8P", "SCHR", "EU8P", "EU8P", "EU8P", "SCHR", "E16"]
M16_PAIRS = [i for i, p in enumerate(PATHS) if p == "E16"]
M8_PAIRS = [i for i, p in enumerate(PATHS) if p in ("SCHR", "EU8P")]
M16_POS = {pr: i for i, pr in enumerate(M16_PAIRS)}
M8_POS = {pr: i for i, pr in enumerate(M8_PAIRS)}
SCHR_BIAS_LIVE = 16256.0 - 134.0
SCHR_M_LIVE = 252
SCHR_M_DEAD = 24
SCHR_MSCALE = SCHR_BIAS_LIVE / SCHR_M_LIVE  # 63.976...: m*this = bias

_NC = None
LAST_RESULT = None   # BassKernelResults of the most recent run (for profiling)


def _build_nc(bpc=BPC, s=S):
    import concourse.bacc as bacc
    import concourse.tile as tile
    from concourse import mybir

    bf16 = mybir.dt.bfloat16
    f32 = mybir.dt.float32
    i16 = mybir.dt.int16
    u8 = mybir.dt.uint8

    # qT is host-pre-scaled by a = 128/(ln2*sqrt(D)), so PSUM scores are
    # already in Schraudolph units; exact exp just rescales by ln2/128
    escale = float(np.log(2.0) / P)
    invs2 = float(1.0 / np.sqrt(2.0))
    mscale = float(SCHR_MSCALE)

    n16 = len(M16_PAIRS) * 2     # k-blocks in m16
    n8 = len(M8_PAIRS) * 2       # k-blocks in m8

    nc = bacc.Bacc()
    qT = nc.declare_dram_parameter("qT", [bpc, P, s], bf16, isOutput=False)
    kT = nc.declare_dram_parameter("kT", [bpc, P, s], bf16, isOutput=False)
    # va host-packed as [p, kb*(D+1)]: row p holds v[kb*128+p, :]+[1] per kb
    va = nc.declare_dram_parameter(
        "va", [bpc, P, KB * (D + 1)], bf16, isOutput=False
    )
    # m16[b, i*2P+p, q] int16, i indexes M16_PAIRS: 0/1 not-mask rows for
    # E16 pairs
    m16 = nc.declare_dram_parameter("m16", [bpc, n16 * P, s], i16, isOutput=False)
    # m8: uint8 rows for SCHR pairs (bias codes {252 live, 24 dead}) and
    # EU8P pairs ({1, 0} multiplier), indexed by M8_PAIRS position
    m8 = nc.declare_dram_parameter("m8", [bpc, n8 * P, s], u8, isOutput=False)
    # out packed [b, qt, p, sl*D+d] so each partition row is one contiguous
    # 1KB chunk (avoids the <512B DMA latency penalty); host unpermutes
    out = nc.declare_dram_parameter(
        "out", [bpc, NQT, P, 4 * D], bf16, isOutput=True
    )

    KCW = [2 * P, 2 * P, 4 * P, 4 * P, 4 * P]
    KCO = [0, 2 * P, 4 * P, 8 * P, 12 * P]

    with tile.TileContext(nc) as tc:
        with (
            tc.tile_pool(name="qk", bufs=3) as qkp,
            tc.tile_pool(name="vp", bufs=3) as vp,
            tc.tile_pool(name="m16p", bufs=M16_BUFS) as m16p,
            tc.tile_pool(name="m8p", bufs=M8_BUFS) as m8p,
            tc.tile_pool(name="attn", bufs=ATT_BUFS) as attnp,
            tc.tile_pool(name="attne", bufs=ATTE_BUFS) as attnep,
            tc.tile_pool(name="schri", bufs=SCHRI_BUFS) as schrip,
            tc.tile_pool(name="outp", bufs=3) as outp,
            tc.tile_pool(name="const", bufs=1) as constp,
            tc.tile_pool(name="rp", bufs=12) as rp,
            tc.tile_pool(name="spsum", bufs=SPSUM_BUFS, space="PSUM") as spsum,
            tc.tile_pool(name="avpsum", bufs=AV_BUFS, space="PSUM") as avpsum,
        ):
            # ACT warm-up: load the Exp table (~1.3us) during initial DMAs
            warm = constp.tile([P, 128], bf16, name="warm", tag="warm")
            nc.vector.memset(warm[:], 0.0)
            nc.scalar.activation(
                warm[:, 0:1], warm[:, 0:1], mybir.ActivationFunctionType.Exp
            )
            # PE warm-up burst: p-state ramps to full clock with sustained
            # busy; burn the initial DMA-wait window on dummy matmuls
            wp = avpsum.tile([P, 2, D + 1], f32, name="warm_mm", tag="av")
            for _ in range(WARM_N):
                nc.tensor.matmul(
                    wp[:, 0, 0:P], lhsT=warm[:], rhs=warm[:],
                    start=True, stop=True,
                )

            # ---- per-batch tiles + loads, deadline-ordered -------------
            kt_ch = [[None] * len(KCW) for _ in range(bpc)]
            qt_ch = [[None] * NQT for _ in range(bpc)]
            va_s = [None] * bpc
            m16_t = [[None] * NQT for _ in range(bpc)]
            m8_t = [[None] * NQT for _ in range(bpc)]

            for b in range(bpc):
                kt_ch[b] = [
                    qkp.tile([P, w], bf16, name="kt_ch", tag=f"kt{i}", bufs=3)
                    for i, w in enumerate(KCW)
                ]
                qt_ch[b] = [
                    qkp.tile([P, QW], bf16, name="qt_ch", tag="qt",
                             bufs=NQT + 2)
                    for _ in range(NQT)
                ]
                va_s[b] = vp.tile([P, KB, D + 1], bf16, name="va_s", tag="va")
                m16_t[b] = [
                    m16p.tile([P, n16, QW], i16, name="m16_t", tag="m16",
                              bufs=M16_BUFS)
                    for _ in range(NQT)
                ]
                m8_t[b] = [
                    m8p.tile([P, n8, QW], u8, name="m8_t", tag="m8",
                             bufs=M8_BUFS)
                    for _ in range(NQT)
                ]

            def _ldk(b, i, eng):
                eng.dma_start(
                    out=kt_ch[b][i][:], in_=kT[b, :, KCO[i]:KCO[i] + KCW[i]]
                )

            def _ldq(b, qt, eng):
                eng.dma_start(
                    out=qt_ch[b][qt][:], in_=qT[b, :, qt * QW:(qt + 1) * QW]
                )

            def _ldva(b, lo, hi, eng):
                vr = va[b, :, :].rearrange("p (kb d) -> p kb d", d=D + 1)
                eng.dma_start(out=va_s[b][:, lo:hi, :], in_=vr[:, lo:hi, :])

            def _ldm16(b, qt, eng):
                m16r = m16[b, :, :].rearrange("(kb p) q -> p kb q", p=P)
                eng.dma_start(
                    out=m16_t[b][qt][:],
                    in_=m16r[:, :, qt * QW:(qt + 1) * QW],
                )

            def _ldm8(b, qt, eng, lo=0, hi=None):
                hi = n8 if hi is None else hi
                m8r = m8[b, :, :].rearrange("(kb p) q -> p kb q", p=P)
                eng.dma_start(
                    out=m8_t[b][qt][:, lo:hi, :],
                    in_=m8r[:, lo:hi, qt * QW:(qt + 1) * QW],
                )

            SP, ACT, POOL = nc.sync, nc.scalar, nc.gpsimd
            # SP and ACT queues have exec depth 0 — a DMA occupies them for
            # its whole transfer (~2.4us for a merged m8). So the head uses
            # the idle ACT/Pool queues for q-tile-0 masks/va, and everything
            # else is issued JUST IN TIME from the pass stream below so SP's
            # serial queue tracks deadlines ~1.5 passes ahead.
            _ldk(0, 0, SP)
            _ldq(0, 0, SP)
            _ldk(0, 1, SP)
            _ldk(0, 2, SP)
            _ldk(0, 3, SP)
            _ldk(0, 4, SP)
            _ldm16(0, 0, ACT)            # E16 pairs 0,7 (pair 0 first)
            _ldm8(0, 0, POOL)            # SWDGE: holds Pool only for gen
            _ldva(0, 0, 8, POOL)
            _ldva(0, 8, KB, POOL)

            # ---- merged global pair stream -----------------------------
            def emit_norm(av_t, o_s, b_, qt_):
                for sl in range(4):
                    avap = av_t[sl // 2][:, sl % 2, :]
                    recip = rp.tile([P, 1], f32, name="recip", tag="recip")
                    nc.vector.reciprocal(recip[:], avap[:, D:D + 1])
                    nc.vector.tensor_scalar_mul(
                        o_s[:, sl, :], avap[:, 0:D], recip[:]
                    )
                nc.sync.dma_start(
                    out=out[b_, qt_, :, :].rearrange(
                        "p (sl d) -> p sl d", d=D
                    ),
                    in_=o_s[:, :, :],
                )

            TOT = bpc * NQT * NPAIR
            atts = [None] * TOT
            avs = {}    # pass index -> (av_t list, o_s)

            for g in range(TOT + PD):
                # just-in-time loads: at mid-pass (g%8==4) issue the q-tile
                # loads for pass p+2 (~1.5-pass lead on SP's serial queue);
                # batch-1 kt/va ride earlier boundaries.
                if g < TOT and g % NPAIR == 4:
                    p2s = [1, 2] if g == 4 else [g // NPAIR + 2]
                    for p2 in p2s:
                        if p2 < bpc * NQT:
                            b2, qt2 = divmod(p2, NQT)
                            _ldq(b2, qt2, SP)
                            _ldm16(b2, qt2, SP)
                            _ldm8(b2, qt2, SP)
                if g == 12:
                    _ldk(1, 0, SP)
                    _ldk(1, 1, SP)
                    _ldk(1, 2, SP)
                if g == 20:
                    _ldk(1, 3, SP)
                    _ldk(1, 4, SP)
                    _ldva(1, 0, 8, POOL)
                    _ldva(1, 8, KB, POOL)
                # AV block of step g-PD first: if QK(g) must wait on its
                # s_ps buffer (exp of g-SPSUM_BUFS still draining), the PE
                # does these matmuls instead of idling.
                gp = g - PD
                if gp >= 0:
                    bp = gp // (NQT * NPAIR)
                    tp = gp % NPAIR
                    pidx = gp // NPAIR
                    if tp == 0:
                        avs[pidx] = (
                            [
                                avpsum.tile(
                                    [P, 2, D + 1], f32, name="av", tag="av"
                                )
                                for _ in range(2)
                            ],
                            outp.tile([P, 4, D], bf16, name="o_s", tag="o_s"),
                        )
                    av_t, o_s = avs[pidx]
                    for h in range(2):
                        kb = 2 * tp + h
                        for sl in range(4):
                            nc.tensor.matmul(
                                av_t[sl // 2][:, sl % 2, :],
                                lhsT=atts[gp][:, h, sl * P:(sl + 1) * P],
                                rhs=va_s[bp][:, kb, :],
                                start=(kb == 0 and sl % 2 == 0),
                                stop=(kb == KB - 1 and sl % 2 == 1),
                            )
                    if tp == NPAIR - 1:
                        qtp = (gp // NPAIR) % NQT
                        emit_norm(av_t, o_s, bp, qtp)
                        del avs[pidx]

                if g < TOT:
                    b = g // (NQT * NPAIR)
                    qt = (g // NPAIR) % NQT
                    t = g % NPAIR
                    s_ps = spsum.tile([P, 2, QW], f32, name="s_ps", tag="s")
                    for h in range(2):
                        kb = 2 * t + h
                        kc = next(
                            i for i in range(len(KCW))
                            if KCO[i] <= kb * P < KCO[i] + KCW[i]
                        )
                        ko = kb * P - KCO[kc]
                        nc.tensor.matmul(
                            s_ps[:, h, :],
                            lhsT=kt_ch[b][kc][:, ko:ko + P],
                            rhs=qt_ch[b][qt][:],
                            start=True,
                            stop=True,
                        )
                    path = PATHS[t]
                    att = attnp.tile([P, 2, QW], bf16, name="att", tag="att")
                    if path == "SCHR":
                        pos = M8_POS[t]
                        att1 = schrip.tile(
                            [P, 2, QW], bf16, name="att1", tag="schri"
                        )
                        att2 = schrip.tile(
                            [P, 2, QW], bf16, name="att2", tag="schri"
                        )
                        # PSUM-reading ops stay on DVE (GPSIMD may not
                        # access PSUM); SBUF-only tail split DVE/Pool
                        nc.vector.scalar_tensor_tensor(
                            out=att1[:].bitcast(i16),
                            in0=m8_t[b][qt][:, 2 * pos:2 * pos + 2, :],
                            scalar=mscale,
                            in1=s_ps[:],
                            op0=mybir.AluOpType.mult,
                            op1=mybir.AluOpType.add,
                        )
                        nc.vector.tensor_scalar(
                            out=att2[:].bitcast(i16),
                            in0=att1[:].bitcast(i16),
                            scalar1=64,
                            scalar2=None,
                            op0=mybir.AluOpType.add,
                        )
                        nc.vector.tensor_scalar(
                            out=att2[:],
                            in0=att2[:],
                            scalar1=invs2,
                            scalar2=None,
                            op0=mybir.AluOpType.mult,
                        )
                        nc.gpsimd.tensor_add(att[:], att2[:], att1[:])
                    else:
                        att_e = attnep.tile(
                            [P, 2, QW], bf16, name="att_e", tag="atte"
                        )
                        nc.scalar.activation(
                            att_e[:],
                            s_ps[:],
                            mybir.ActivationFunctionType.Exp,
                            scale=escale,
                        )
                        if path == "E16":
                            pos = M16_POS[t]
                            nc.vector.tensor_mul(
                                att[:], att_e[:],
                                m16_t[b][qt][:, 2 * pos:2 * pos + 2, :],
                            )
                        else:  # EU8P
                            pos = M8_POS[t]
                            nc.gpsimd.tensor_mul(
                                att[:], att_e[:],
                                m8_t[b][qt][:, 2 * pos:2 * pos + 2, :],
                            )
                    atts[g] = att
    nc.compile()
    return nc


def _prep_inputs(q, k, v, mask):
    bf = ml_dtypes.bfloat16
    ones = np.ones((B, S, 1), dtype=np.float32)
    va_full = (
        np.concatenate([np.asarray(v, np.float32), ones], axis=2)
        .reshape(B, S // P, P, D + 1)
        .transpose(0, 2, 1, 3)
        .reshape(B, P, (S // P) * (D + 1))
        .astype(bf)
    )
    ascale = np.float32(P / np.log(2.0) / np.sqrt(D))
    qT_full = np.ascontiguousarray(
        np.asarray(q, np.float32).transpose(0, 2, 1) * ascale
    ).astype(bf)
    kT_full = np.ascontiguousarray(
        np.asarray(k, np.float32).transpose(0, 2, 1)
    ).astype(bf)
    mT = np.asarray(mask, bool).transpose(0, 2, 1)  # [b, k, q]
    m16_full = np.empty((B, len(M16_PAIRS) * 2 * P, S), dtype=np.int16)
    for i, pr in enumerate(M16_PAIRS):
        blk = mT[:, 2 * pr * P:(2 * pr + 2) * P, :]
        r0, r1 = 2 * i * P, (2 * i + 2) * P
        m16_full[:, r0:r1, :] = np.where(blk, np.int16(0), np.int16(1))
    m8_full = np.empty((B, len(M8_PAIRS) * 2 * P, S), dtype=np.uint8)
    for i, pr in enumerate(M8_PAIRS):
        blk = mT[:, 2 * pr * P:(2 * pr + 2) * P, :]
        r0, r1 = 2 * i * P, (2 * i + 2) * P
        if PATHS[pr] == "SCHR":
            m8_full[:, r0:r1, :] = np.where(
                blk, np.uint8(SCHR_M_DEAD), np.uint8(SCHR_M_LIVE)
            )
        else:
            m8_full[:, r0:r1, :] = (~blk).astype(np.uint8)
    return qT_full, kT_full, va_full, m16_full, m8_full


def kernel(q, k, v, mask, _trace=False, _trace_kwargs=None):
    global _NC, LAST_RESULT
    from concourse.bass_utils import run_bass_kernel_spmd

    if _NC is None:
        _NC = _build_nc()

    qT_full, kT_full, va_full, m16_full, m8_full = _prep_inputs(q, k, v, mask)

    in_maps = []
    for c in range(N_CORES):
        lo, hi = c * BPC, (c + 1) * BPC
        in_maps.append(
            {
                "qT": qT_full[lo:hi],
                "kT": kT_full[lo:hi],
                "va": va_full[lo:hi],
                "m16": m16_full[lo:hi],
                "m8": m8_full[lo:hi],
            }
        )

    kw = {}
    if _trace:
        kw["trace"] = True
        if _trace_kwargs:
            kw.update(_trace_kwargs)
    LAST_RESULT = run_bass_kernel_spmd(_NC, in_maps, list(range(N_CORES)), **kw)
    res = LAST_RESULT.results
    outp = np.concatenate(
        [np.asarray(res[c]["out"]) for c in range(N_CORES)], axis=0
    ).astype(np.float32)
    # [b, qt, p, sl*D+d] -> [b, qt*512 + sl*128 + p, d]
    out = (
        outp.reshape(B, NQT, P, 4, D)
        .transpose(0, 1, 3, 2, 4)
        .reshape(B, S, D)
    )
    return np.ascontiguousarray(out)


# revision 20
# speedup vs baseline: 1.0645x; 1.0106x over previous
"""Masked attention on 8 TRN2 NeuronCores — pure data-parallel over batch.

Full inputs:  q,k,v (16,2048,128) f32, mask (16,2048,2048) bool.
Output:       (16,2048,128) f32.

Per core (2 batches). Scores are computed transposed S^T[k,q] = K·Q^T in bf16
on the PE so the AV contraction lands on the partition axis. The exp+mask
elementwise pass (8.4M elems/core) is split across engines by k-block pair
(SCHR pair-averaged Schraudolph on DVE/Pool, exact ACT exp + mask-mul on
DVE (E16) or Pool (EU8P)) exactly as before, but the schedule is a single
SOFTWARE-PIPELINED GLOBAL STREAM over all 64 (batch, q-tile, pair) steps:

  - Per step g: QK(g) matmuls, elementwise(g), then the AV matmuls of step
    g-PD. AV lags globally, so a pass's trailing AV blocks interleave with
    the NEXT pass's QK matmuls in the PE queue instead of serializing the
    pass boundary (the baseline lost ~3.1us/pass to this).
  - Mask DMAs are merged to one m8 + one m16 load per q-tile (the HWDGE
    descriptor-generator is a single ~625ns/DMA device shared by SP/ACT/DVE
    queues; at 94 DMAs it was a 59us serial resource, now ~48 DMAs).
  - All loads for both batches are issued up-front in deadline order on SP;
    the first q-tile's masks ride ACT/DVE queues so the head doesn't wait
    on SP's ~650ns/DMA issue rate. Stores stay on SP — they sit after all
    loads in SP's in-order queue, so their norm-waits can't block a load.
  - AV accumulates [q,129] per slot with a ones-column in va giving the
    softmax denominator; normalization is DVE reciprocal + scale, stored
    as [q, d] rows from SP's queue (host unpermutes).
"""

import numpy as np
import ml_dtypes

B, S, D = 16, 2048, 128
N_CORES = 8
BPC = B // N_CORES   # batches per core
P = 128              # partitions
QW = 512             # q-tile width
NQT = S // QW        # q-tiles per batch
KB = S // P          # k-blocks per batch
NPAIR = KB // 2      # k-block pairs per q-tile pass
PD = 6               # AV matmuls pipelined this many pairs behind elementwise
SPSUM_BUFS = 3       # scores double/triple buffering (2 banks each)
AV_BUFS = 2          # AV accumulator tiles (1 bank each)
ATT_BUFS = 10
ATTE_BUFS = 12    # att_e (exp outputs); separate pool from SCHR intermediates
SCHRI_BUFS = 8    # att1/att2 SCHR intermediates
WARM_N = 14
M8_BUFS = 5
M16_BUFS = 5

# Per-pair elementwise path (8 entries, k-block pairs 0..7):
#   SCHR: pair-averaged Schraudolph, DVE stt + DVE/Pool tail (m8 bias codes)
#   E16:  ACT exp + DVE tensor_mul (m16 0/1 int16 rows)
#   EU8P: ACT exp + Pool stt-mul (m8 uint8 rows)
# Order within a pass: shortest-latency paths (E16) at the pair positions
# with the earliest/latest AV deadlines; the long SCHR chains sit at {2,6}
# so ACT's six serialized exps never run more than 3 back-to-back (exp
# completion spacing 1.038us vs QK spacing 0.857us — longer runs make the
# s_ps WAR bite), and their AV deadline (pair index + PD) has slack.
PATHS = ["E16", "EU8P", "SCHR", "EU8P", "EU8P", "EU8P", "SCHR", "E16"]
M16_PAIRS = [i for i, p in enumerate(PATHS) if p == "E16"]
M8_PAIRS = [i for i, p in enumerate(PATHS) if p in ("SCHR", "EU8P")]
M16_POS = {pr: i for i, pr in enumerate(M16_PAIRS)}
M8_POS = {pr: i for i, pr in enumerate(M8_PAIRS)}
SCHR_BIAS_LIVE = 16256.0 - 134.0
SCHR_M_LIVE = 252
SCHR_M_DEAD = 24
SCHR_MSCALE = SCHR_BIAS_LIVE / SCHR_M_LIVE  # 63.976...: m*this = bias

_NC = None
LAST_RESULT = None   # BassKernelResults of the most recent run (for profiling)


def _build_nc(bpc=BPC, s=S):
    import concourse.bacc as bacc
    import concourse.tile as tile
    from concourse import mybir

    bf16 = mybir.dt.bfloat16
    f32 = mybir.dt.float32
    i16 = mybir.dt.int16
    u8 = mybir.dt.uint8

    # qT is host-pre-scaled by a = 128/(ln2*sqrt(D)), so PSUM scores are
    # already in Schraudolph units; exact exp just rescales by ln2/128
    escale = float(np.log(2.0) / P)
    invs2 = float(1.0 / np.sqrt(2.0))
    mscale = float(SCHR_MSCALE)

    n16 = len(M16_PAIRS) * 2     # k-blocks in m16
    n8 = len(M8_PAIRS) * 2       # k-blocks in m8

    nc = bacc.Bacc()
    qT = nc.declare_dram_parameter("qT", [bpc, P, s], bf16, isOutput=False)
    kT = nc.declare_dram_parameter("kT", [bpc, P, s], bf16, isOutput=False)
    # va host-packed as [p, kb*(D+1)]: row p holds v[kb*128+p, :]+[1] per kb
    va = nc.declare_dram_parameter(
        "va", [bpc, P, KB * (D + 1)], bf16, isOutput=False
    )
    # m16[b, i*2P+p, q] int16, i indexes M16_PAIRS: 0/1 not-mask rows for
    # E16 pairs
    m16 = nc.declare_dram_parameter("m16", [bpc, n16 * P, s], i16, isOutput=False)
    # m8: uint8 rows for SCHR pairs (bias codes {252 live, 24 dead}) and
    # EU8P pairs ({1, 0} multiplier), indexed by M8_PAIRS position
    m8 = nc.declare_dram_parameter("m8", [bpc, n8 * P, s], u8, isOutput=False)
    # out packed [b, qt, p, sl*D+d] so each partition row is one contiguous
    # 1KB chunk (avoids the <512B DMA latency penalty); host unpermutes
    out = nc.declare_dram_parameter(
        "out", [bpc, NQT, P, 4 * D], bf16, isOutput=True
    )

    KCW = [2 * P, 2 * P, 4 * P, 4 * P, 4 * P]
    KCO = [0, 2 * P, 4 * P, 8 * P, 12 * P]

    with tile.TileContext(nc) as tc:
        with (
            tc.tile_pool(name="qk", bufs=3) as qkp,
            tc.tile_pool(name="vp", bufs=3) as vp,
            tc.tile_pool(name="m16p", bufs=M16_BUFS) as m16p,
            tc.tile_pool(name="m8p", bufs=M8_BUFS) as m8p,
            tc.tile_pool(name="attn", bufs=ATT_BUFS) as attnp,
            tc.tile_pool(name="attne", bufs=ATTE_BUFS) as attnep,
            tc.tile_pool(name="schri", bufs=SCHRI_BUFS) as schrip,
            tc.tile_pool(name="outp", bufs=3) as outp,
            tc.tile_pool(name="const", bufs=1) as constp,
            tc.tile_pool(name="rp", bufs=12) as rp,
            tc.tile_pool(name="spsum", bufs=SPSUM_BUFS, space="PSUM") as spsum,
            tc.tile_pool(name="avpsum", bufs=AV_BUFS, space="PSUM") as avpsum,
        ):
            # ACT warm-up: load the Exp table (~1.3us) during initial DMAs.
            # Writes its own scratch tile so the PE warm-ups below depend
            # only on the DVE memset, not on ACT's serial table load.
            warm = constp.tile([P, 128], bf16, name="warm", tag="warm")
            wact = constp.tile([P, 1], bf16, name="wact", tag="wact")
            nc.vector.memset(warm[:], 0.0)
            nc.vector.memset(wact[:], 0.0)
            nc.scalar.activation(
                wact[:], wact[:], mybir.ActivationFunctionType.Exp
            )
            # PE warm-up burst: p-state ramps to full clock with sustained
            # busy; burn the initial DMA-wait window on dummy matmuls
            wp = avpsum.tile([P, 2, D + 1], f32, name="warm_mm", tag="av")
            for _ in range(WARM_N):
                nc.tensor.matmul(
                    wp[:, 0, 0:P], lhsT=warm[:], rhs=warm[:],
                    start=True, stop=True,
                )

            # ---- per-batch tiles + loads, deadline-ordered -------------
            kt_ch = [[None] * len(KCW) for _ in range(bpc)]
            qt_ch = [[None] * NQT for _ in range(bpc)]
            va_s = [None] * bpc
            m16_t = [[None] * NQT for _ in range(bpc)]
            m8_t = [[None] * NQT for _ in range(bpc)]

            for b in range(bpc):
                kt_ch[b] = [
                    qkp.tile([P, w], bf16, name="kt_ch", tag=f"kt{i}", bufs=3)
                    for i, w in enumerate(KCW)
                ]
                qt_ch[b] = [
                    qkp.tile([P, QW], bf16, name="qt_ch", tag="qt",
                             bufs=NQT + 2)
                    for _ in range(NQT)
                ]
                va_s[b] = vp.tile([P, KB, D + 1], bf16, name="va_s", tag="va")
                m16_t[b] = [
                    m16p.tile([P, n16, QW], i16, name="m16_t", tag="m16",
                              bufs=M16_BUFS)
                    for _ in range(NQT)
                ]
                m8_t[b] = [
                    m8p.tile([P, n8, QW], u8, name="m8_t", tag="m8",
                             bufs=M8_BUFS)
                    for _ in range(NQT)
                ]

            def _ldk(b, i, eng):
                eng.dma_start(
                    out=kt_ch[b][i][:], in_=kT[b, :, KCO[i]:KCO[i] + KCW[i]]
                )

            def _ldq(b, qt, eng):
                eng.dma_start(
                    out=qt_ch[b][qt][:], in_=qT[b, :, qt * QW:(qt + 1) * QW]
                )

            def _ldva(b, lo, hi, eng):
                vr = va[b, :, :].rearrange("p (kb d) -> p kb d", d=D + 1)
                eng.dma_start(out=va_s[b][:, lo:hi, :], in_=vr[:, lo:hi, :])

            def _ldm16(b, qt, eng):
                m16r = m16[b, :, :].rearrange("(kb p) q -> p kb q", p=P)
                eng.dma_start(
                    out=m16_t[b][qt][:],
                    in_=m16r[:, :, qt * QW:(qt + 1) * QW],
                )

            def _ldm8(b, qt, eng, lo=0, hi=None):
                hi = n8 if hi is None else hi
                m8r = m8[b, :, :].rearrange("(kb p) q -> p kb q", p=P)
                eng.dma_start(
                    out=m8_t[b][qt][:, lo:hi, :],
                    in_=m8r[:, lo:hi, qt * QW:(qt + 1) * QW],
                )

            SP, ACT, POOL = nc.sync, nc.scalar, nc.gpsimd
            # SP and ACT queues have exec depth 0 — a DMA occupies them for
            # its whole transfer (~2.4us for a merged m8). So the head uses
            # the idle ACT/Pool queues for q-tile-0 masks/va, and everything
            # else is issued JUST IN TIME from the pass stream below so SP's
            # serial queue tracks deadlines ~1.5 passes ahead.
            _ldk(0, 0, SP)
            _ldq(0, 0, SP)
            _ldk(0, 1, SP)
            _ldk(0, 2, SP)
            _ldk(0, 3, SP)
            _ldk(0, 4, SP)
            _ldm16(0, 0, ACT)            # E16 pairs 0,7 (pair 0 first)
            _ldm8(0, 0, POOL)            # SWDGE: holds Pool only for gen
            _ldva(0, 0, 8, POOL)
            _ldva(0, 8, KB, POOL)

            # ---- merged global pair stream -----------------------------
            def emit_norm(av_t, o_s, b_, qt_):
                for sl in range(4):
                    avap = av_t[sl // 2][:, sl % 2, :]
                    recip = rp.tile([P, 1], f32, name="recip", tag="recip")
                    nc.vector.reciprocal(recip[:], avap[:, D:D + 1])
                    nc.vector.tensor_scalar_mul(
                        o_s[:, sl, :], avap[:, 0:D], recip[:]
                    )
                nc.sync.dma_start(
                    out=out[b_, qt_, :, :].rearrange(
                        "p (sl d) -> p sl d", d=D
                    ),
                    in_=o_s[:, :, :],
                )

            TOT = bpc * NQT * NPAIR
            atts = [None] * TOT
            avs = {}    # pass index -> (av_t list, o_s)

            for g in range(TOT + PD):
                # just-in-time loads: at mid-pass (g%8==4) issue the q-tile
                # loads for pass p+2 (~1.5-pass lead on SP's serial queue);
                # batch-1 kt/va ride earlier boundaries.
                if g < TOT and g % NPAIR == 4:
                    p2s = [1, 2] if g == 4 else [g // NPAIR + 2]
                    for p2 in p2s:
                        if p2 < bpc * NQT:
                            b2, qt2 = divmod(p2, NQT)
                            _ldq(b2, qt2, SP)
                            _ldm16(b2, qt2, SP)
                            _ldm8(b2, qt2, SP)
                if g == 12:
                    _ldk(1, 0, SP)
                    _ldk(1, 1, SP)
                    _ldk(1, 2, SP)
                if g == 20:
                    _ldk(1, 3, SP)
                    _ldk(1, 4, SP)
                    _ldva(1, 0, 8, POOL)
                    _ldva(1, 8, KB, POOL)
                # AV block of step g-PD first: if QK(g) must wait on its
                # s_ps buffer (exp of g-SPSUM_BUFS still draining), the PE
                # does these matmuls instead of idling.
                gp = g - PD
                if gp >= 0:
                    bp = gp // (NQT * NPAIR)
                    tp = gp % NPAIR
                    pidx = gp // NPAIR
                    if tp == 0:
                        avs[pidx] = (
                            [
                                avpsum.tile(
                                    [P, 2, D + 1], f32, name="av", tag="av"
                                )
                                for _ in range(2)
                            ],
                            outp.tile([P, 4, D], bf16, name="o_s", tag="o_s"),
                        )
                    av_t, o_s = avs[pidx]
                    for h in range(2):
                        kb = 2 * tp + h
                        for sl in range(4):
                            nc.tensor.matmul(
                                av_t[sl // 2][:, sl % 2, :],
                                lhsT=atts[gp][:, h, sl * P:(sl + 1) * P],
                                rhs=va_s[bp][:, kb, :],
                                start=(kb == 0 and sl % 2 == 0),
                                stop=(kb == KB - 1 and sl % 2 == 1),
                            )
                    if tp == NPAIR - 1:
                        qtp = (gp // NPAIR) % NQT
                        emit_norm(av_t, o_s, bp, qtp)
                        del avs[pidx]

                if g < TOT:
                    b = g // (NQT * NPAIR)
                    qt = (g // NPAIR) % NQT
                    t = g % NPAIR
                    s_ps = spsum.tile([P, 2, QW], f32, name="s_ps", tag="s")
                    for h in range(2):
                        kb = 2 * t + h
                        kc = next(
                            i for i in range(len(KCW))
                            if KCO[i] <= kb * P < KCO[i] + KCW[i]
                        )
                        ko = kb * P - KCO[kc]
                        nc.tensor.matmul(
                            s_ps[:, h, :],
                            lhsT=kt_ch[b][kc][:, ko:ko + P],
                            rhs=qt_ch[b][qt][:],
                            start=True,
                            stop=True,
                        )
                    path = PATHS[t]
                    att = attnp.tile([P, 2, QW], bf16, name="att", tag="att")
                    if path == "SCHR":
                        pos = M8_POS[t]
                        att1 = schrip.tile(
                            [P, 2, QW], bf16, name="att1", tag="schri"
                        )
                        att2 = schrip.tile(
                            [P, 2, QW], bf16, name="att2", tag="schri"
                        )
                        # PSUM-reading ops stay on DVE (GPSIMD may not
                        # access PSUM); SBUF-only tail split DVE/Pool
                        nc.vector.scalar_tensor_tensor(
                            out=att1[:].bitcast(i16),
                            in0=m8_t[b][qt][:, 2 * pos:2 * pos + 2, :],
                            scalar=mscale,
                            in1=s_ps[:],
                            op0=mybir.AluOpType.mult,
                            op1=mybir.AluOpType.add,
                        )
                        nc.vector.tensor_scalar(
                            out=att2[:].bitcast(i16),
                            in0=att1[:].bitcast(i16),
                            scalar1=64,
                            scalar2=None,
                            op0=mybir.AluOpType.add,
                        )
                        nc.vector.tensor_scalar(
                            out=att2[:],
                            in0=att2[:],
                            scalar1=invs2,
                            scalar2=None,
                            op0=mybir.AluOpType.mult,
                        )
                        nc.gpsimd.tensor_add(att[:], att2[:], att1[:])
                    else:
                        att_e = attnep.tile(
                            [P, 2, QW], bf16, name="att_e", tag="atte"
                        )
                        nc.scalar.activation(
                            att_e[:],
                            s_ps[:],
                            mybir.ActivationFunctionType.Exp,
                            scale=escale,
                        )
                        if path == "E16":
                            pos = M16_POS[t]
                            nc.vector.tensor_mul(
                                att[:], att_e[:],
                                m16_t[b][qt][:, 2 * pos:2 * pos + 2, :],
                            )
                        else:  # EU8P
                            pos = M8_POS[t]
                            nc.gpsimd.tensor_mul(
                                att[:], att_e[:],
                                m8_t[b][qt][:, 2 * pos:2 * pos + 2, :],
                            )
                    atts[g] = att
    nc.compile()
    return nc


def _prep_inputs(q, k, v, mask):
    bf = ml_dtypes.bfloat16
    ones = np.ones((B, S, 1), dtype=np.float32)
    va_full = (
        np.concatenate([np.asarray(v, np.float32), ones], axis=2)
        .reshape(B, S // P, P, D + 1)
        .transpose(0, 2, 1, 3)
        .reshape(B, P, (S // P) * (D + 1))
        .astype(bf)
    )
    ascale = np.float32(P / np.log(2.0) / np.sqrt(D))
    qT_full = np.ascontiguousarray(
        np.asarray(q, np.float32).transpose(0, 2, 1) * ascale
    ).astype(bf)
    kT_full = np.ascontiguousarray(
        np.asarray(k, np.float32).transpose(0, 2, 1)
    ).astype(bf)
    mT = np.asarray(mask, bool).transpose(0, 2, 1)  # [b, k, q]
    m16_full = np.empty((B, len(M16_PAIRS) * 2 * P, S), dtype=np.int16)
    for i, pr in enumerate(M16_PAIRS):
        blk = mT[:, 2 * pr * P:(2 * pr + 2) * P, :]
        r0, r1 = 2 * i * P, (2 * i + 2) * P
        m16_full[:, r0:r1, :] = np.where(blk, np.int16(0), np.int16(1))
    m8_full = np.empty((B, len(M8_PAIRS) * 2 * P, S), dtype=np.uint8)
    for i, pr in enumerate(M8_PAIRS):
        blk = mT[:, 2 * pr * P:(2 * pr + 2) * P, :]
        r0, r1 = 2 * i * P, (2 * i + 2) * P
        if PATHS[pr] == "SCHR":
            m8_full[:, r0:r1, :] = np.where(
                blk, np.uint8(SCHR_M_DEAD), np.uint8(SCHR_M_LIVE)
            )
        else:
            m8_full[:, r0:r1, :] = (~blk).astype(np.uint8)
    return qT_full, kT_full, va_full, m16_full, m8_full


def kernel(q, k, v, mask, _trace=False, _trace_kwargs=None):
    global _NC, LAST_RESULT
    from concourse.bass_utils import run_bass_kernel_spmd

    if _NC is None:
        _NC = _build_nc()

    qT_full, kT_full, va_full, m16_full, m8_full = _prep_inputs(q, k, v, mask)

    in_maps = []
    for c in range(N_CORES):
        lo, hi = c * BPC, (c + 1) * BPC
        in_maps.append(
            {
                "qT": qT_full[lo:hi],
                "kT": kT_full[lo:hi],
                "va": va_full[lo:hi],
                "m16": m16_full[lo:hi],
                "m8": m8_full[lo:hi],
            }
        )

    kw = {}
    if _trace:
        kw["trace"] = True
        if _trace_kwargs:
            kw.update(_trace_kwargs)
    LAST_RESULT = run_bass_kernel_spmd(_NC, in_maps, list(range(N_CORES)), **kw)
    res = LAST_RESULT.results
    outp = np.concatenate(
        [np.asarray(res[c]["out"]) for c in range(N_CORES)], axis=0
    ).astype(np.float32)
    # [b, qt, p, sl*D+d] -> [b, qt*512 + sl*128 + p, d]
    out = (
        outp.reshape(B, NQT, P, 4, D)
        .transpose(0, 1, 3, 2, 4)
        .reshape(B, S, D)
    )
    return np.ascontiguousarray(out)


# revision 23
# speedup vs baseline: 1.0665x; 1.0019x over previous
"""Masked attention on 8 TRN2 NeuronCores — pure data-parallel over batch.

Full inputs:  q,k,v (16,2048,128) f32, mask (16,2048,2048) bool.
Output:       (16,2048,128) f32.

Per core (2 batches). Scores are computed transposed S^T[k,q] = K·Q^T in bf16
on the PE so the AV contraction lands on the partition axis. The exp+mask
elementwise pass (8.4M elems/core) is split across engines by k-block pair
(SCHR pair-averaged Schraudolph on DVE/Pool, exact ACT exp + mask-mul on
DVE (E16) or Pool (EU8P)) exactly as before, but the schedule is a single
SOFTWARE-PIPELINED GLOBAL STREAM over all 64 (batch, q-tile, pair) steps:

  - Per step g: QK(g) matmuls, elementwise(g), then the AV matmuls of step
    g-PD. AV lags globally, so a pass's trailing AV blocks interleave with
    the NEXT pass's QK matmuls in the PE queue instead of serializing the
    pass boundary (the baseline lost ~3.1us/pass to this).
  - Mask DMAs are merged to one m8 + one m16 load per q-tile (the HWDGE
    descriptor-generator is a single ~625ns/DMA device shared by SP/ACT/DVE
    queues; at 94 DMAs it was a 59us serial resource, now ~48 DMAs).
  - All loads for both batches are issued up-front in deadline order on SP;
    the first q-tile's masks ride ACT/DVE queues so the head doesn't wait
    on SP's ~650ns/DMA issue rate. Stores stay on SP — they sit after all
    loads in SP's in-order queue, so their norm-waits can't block a load.
  - AV accumulates [q,129] per slot with a ones-column in va giving the
    softmax denominator; normalization is DVE reciprocal + scale, stored
    as [q, d] rows from SP's queue (host unpermutes).
"""

import numpy as np
import ml_dtypes

B, S, D = 16, 2048, 128
N_CORES = 8
BPC = B // N_CORES   # batches per core
P = 128              # partitions
QW = 512             # q-tile width
NQT = S // QW        # q-tiles per batch
KB = S // P          # k-blocks per batch
NPAIR = KB // 2      # k-block pairs per q-tile pass
PD = 6               # AV matmuls pipelined this many pairs behind elementwise
SPSUM_BUFS = 3       # scores double/triple buffering (2 banks each)
AV_BUFS = 2          # AV accumulator tiles (1 bank each)
ATT_BUFS = 10
ATTE_BUFS = 12    # att_e (exp outputs); separate pool from SCHR intermediates
SCHRI_BUFS = 8    # att1/att2 SCHR intermediates
WARM_N = 14
M8_BUFS = 5
M16_BUFS = 5

# Per-pair elementwise path (8 entries, k-block pairs 0..7):
#   SCHR: pair-averaged Schraudolph, DVE stt + DVE/Pool tail (m8 bias codes)
#   E16:  ACT exp + DVE tensor_mul (m16 0/1 int16 rows)
#   EU8P: ACT exp + Pool stt-mul (m8 uint8 rows)
# Order within a pass: shortest-latency paths (E16) at the pair positions
# with the earliest/latest AV deadlines; the long SCHR chains sit at {2,6}
# so ACT's six serialized exps never run more than 3 back-to-back (exp
# completion spacing 1.038us vs QK spacing 0.857us — longer runs make the
# s_ps WAR bite), and their AV deadline (pair index + PD) has slack.
PATHS = ["E16", "EU8P", "SCHR", "EU8P", "EU8P", "EU8P", "SCHR", "E16"]
M16_PAIRS = [i for i, p in enumerate(PATHS) if p == "E16"]
M8_PAIRS = [i for i, p in enumerate(PATHS) if p in ("SCHR", "EU8P")]
M16_POS = {pr: i for i, pr in enumerate(M16_PAIRS)}
M8_POS = {pr: i for i, pr in enumerate(M8_PAIRS)}
SCHR_BIAS_LIVE = 16256.0 - 134.0
SCHR_M_LIVE = 252
SCHR_M_DEAD = 24
SCHR_MSCALE = SCHR_BIAS_LIVE / SCHR_M_LIVE  # 63.976...: m*this = bias

_NC = None
LAST_RESULT = None   # BassKernelResults of the most recent run (for profiling)


def _build_nc(bpc=BPC, s=S):
    import concourse.bacc as bacc
    import concourse.tile as tile
    from concourse import mybir

    bf16 = mybir.dt.bfloat16
    f32 = mybir.dt.float32
    i16 = mybir.dt.int16
    u8 = mybir.dt.uint8

    # qT is host-pre-scaled by a = 128/(ln2*sqrt(D)), so PSUM scores are
    # already in Schraudolph units; exact exp just rescales by ln2/128
    escale = float(np.log(2.0) / P)
    invs2 = float(1.0 / np.sqrt(2.0))
    mscale = float(SCHR_MSCALE)

    n16 = len(M16_PAIRS) * 2     # k-blocks in m16
    n8 = len(M8_PAIRS) * 2       # k-blocks in m8

    nc = bacc.Bacc()
    qT = nc.declare_dram_parameter("qT", [bpc, P, s], bf16, isOutput=False)
    kT = nc.declare_dram_parameter("kT", [bpc, P, s], bf16, isOutput=False)
    # va host-packed as [p, kb*(D+1)]: row p holds v[kb*128+p, :]+[1] per kb
    va = nc.declare_dram_parameter(
        "va", [bpc, P, KB * (D + 1)], bf16, isOutput=False
    )
    # m16[b, i*2P+p, q] int16, i indexes M16_PAIRS: 0/1 not-mask rows for
    # E16 pairs
    m16 = nc.declare_dram_parameter("m16", [bpc, n16 * P, s], i16, isOutput=False)
    # m8: uint8 rows for SCHR pairs (bias codes {252 live, 24 dead}) and
    # EU8P pairs ({1, 0} multiplier), indexed by M8_PAIRS position
    m8 = nc.declare_dram_parameter("m8", [bpc, n8 * P, s], u8, isOutput=False)
    # out packed [b, qt, p, sl*D+d] so each partition row is one contiguous
    # 1KB chunk (avoids the <512B DMA latency penalty); host unpermutes
    out = nc.declare_dram_parameter(
        "out", [bpc, NQT, P, 4 * D], bf16, isOutput=True
    )

    KCW = [2 * P, 2 * P, 4 * P, 4 * P, 4 * P]
    KCO = [0, 2 * P, 4 * P, 8 * P, 12 * P]

    with tile.TileContext(nc) as tc:
        with (
            tc.tile_pool(name="qk", bufs=3) as qkp,
            tc.tile_pool(name="vp", bufs=3) as vp,
            tc.tile_pool(name="m16p", bufs=M16_BUFS) as m16p,
            tc.tile_pool(name="m8p", bufs=M8_BUFS) as m8p,
            tc.tile_pool(name="attn", bufs=ATT_BUFS) as attnp,
            tc.tile_pool(name="attne", bufs=ATTE_BUFS) as attnep,
            tc.tile_pool(name="schri", bufs=SCHRI_BUFS) as schrip,
            tc.tile_pool(name="outp", bufs=3) as outp,
            tc.tile_pool(name="const", bufs=1) as constp,
            tc.tile_pool(name="rp", bufs=12) as rp,
            tc.tile_pool(name="spsum", bufs=SPSUM_BUFS, space="PSUM") as spsum,
            tc.tile_pool(name="avpsum", bufs=AV_BUFS, space="PSUM") as avpsum,
        ):
            # ACT warm-up: load the Exp table (~1.3us) during initial DMAs.
            # Writes its own scratch tile so the PE warm-ups below depend
            # only on the DVE memset, not on ACT's serial table load.
            warm = constp.tile([P, 128], bf16, name="warm", tag="warm")
            wact = constp.tile([P, 1], bf16, name="wact", tag="wact")
            nc.vector.memset(warm[:], 0.0)
            nc.vector.memset(wact[:], 0.0)
            nc.scalar.activation(
                wact[:], wact[:], mybir.ActivationFunctionType.Exp
            )
            # PE warm-up burst: p-state ramps to full clock with sustained
            # busy; burn the initial DMA-wait window on dummy matmuls
            wp = avpsum.tile([P, 2, D + 1], f32, name="warm_mm", tag="av")
            for _ in range(WARM_N):
                nc.tensor.matmul(
                    wp[:, 0, 0:P], lhsT=warm[:], rhs=warm[:],
                    start=True, stop=True,
                )

            # ---- per-batch tiles + loads, deadline-ordered -------------
            kt_ch = [[None] * len(KCW) for _ in range(bpc)]
            qt_ch = [[None] * NQT for _ in range(bpc)]
            va_s = [None] * bpc
            m16_t = [[None] * NQT for _ in range(bpc)]
            m8_t = [[None] * NQT for _ in range(bpc)]

            for b in range(bpc):
                kt_ch[b] = [
                    qkp.tile([P, w], bf16, name="kt_ch", tag=f"kt{i}", bufs=3)
                    for i, w in enumerate(KCW)
                ]
                qt_ch[b] = [
                    qkp.tile([P, QW], bf16, name="qt_ch", tag="qt",
                             bufs=NQT + 2)
                    for _ in range(NQT)
                ]
                va_s[b] = vp.tile([P, KB, D + 1], bf16, name="va_s", tag="va")
                m16_t[b] = [
                    m16p.tile([P, n16, QW], i16, name="m16_t", tag="m16",
                              bufs=M16_BUFS)
                    for _ in range(NQT)
                ]
                m8_t[b] = [
                    m8p.tile([P, n8, QW], u8, name="m8_t", tag="m8",
                             bufs=M8_BUFS)
                    for _ in range(NQT)
                ]

            def _ldk(b, i, eng):
                eng.dma_start(
                    out=kt_ch[b][i][:], in_=kT[b, :, KCO[i]:KCO[i] + KCW[i]]
                )

            def _ldq(b, qt, eng):
                eng.dma_start(
                    out=qt_ch[b][qt][:], in_=qT[b, :, qt * QW:(qt + 1) * QW]
                )

            def _ldva(b, lo, hi, eng):
                vr = va[b, :, :].rearrange("p (kb d) -> p kb d", d=D + 1)
                eng.dma_start(out=va_s[b][:, lo:hi, :], in_=vr[:, lo:hi, :])

            def _ldm16(b, qt, eng):
                m16r = m16[b, :, :].rearrange("(kb p) q -> p kb q", p=P)
                eng.dma_start(
                    out=m16_t[b][qt][:],
                    in_=m16r[:, :, qt * QW:(qt + 1) * QW],
                )

            def _ldm8(b, qt, eng, lo=0, hi=None):
                hi = n8 if hi is None else hi
                m8r = m8[b, :, :].rearrange("(kb p) q -> p kb q", p=P)
                eng.dma_start(
                    out=m8_t[b][qt][:, lo:hi, :],
                    in_=m8r[:, lo:hi, qt * QW:(qt + 1) * QW],
                )

            SP, ACT, POOL = nc.sync, nc.scalar, nc.gpsimd
            # SP and ACT queues have exec depth 0 — a DMA occupies them for
            # its whole transfer (~2.4us for a merged m8). So the head uses
            # the idle ACT/Pool queues for q-tile-0 masks/va, and everything
            # else is issued JUST IN TIME from the pass stream below so SP's
            # serial queue tracks deadlines ~1.5 passes ahead.
            _ldk(0, 0, SP)
            _ldq(0, 0, SP)
            _ldk(0, 1, SP)
            _ldk(0, 2, SP)
            _ldk(0, 3, SP)
            _ldk(0, 4, SP)
            _ldm16(0, 0, ACT)            # E16 pairs 0,7 (pair 0 first)
            _ldm8(0, 0, POOL)            # SWDGE: holds Pool only for gen
            _ldva(0, 0, 8, POOL)
            _ldva(0, 8, KB, POOL)

            # ---- merged global pair stream -----------------------------
            def emit_norm(av_t, o_s, b_, qt_, split=False):
                hbm = out[b_, qt_, :, :].rearrange("p (sl d) -> p sl d", d=D)
                for sl in range(4):
                    avap = av_t[sl // 2][:, sl % 2, :]
                    recip = rp.tile([P, 1], f32, name="recip", tag="recip")
                    nc.vector.reciprocal(recip[:], avap[:, D:D + 1])
                    nc.vector.tensor_scalar_mul(
                        o_s[:, sl, :], avap[:, 0:D], recip[:]
                    )
                    if split and sl == 1:
                        nc.sync.dma_start(
                            out=hbm[:, 0:2, :], in_=o_s[:, 0:2, :]
                        )
                if split:
                    nc.sync.dma_start(out=hbm[:, 2:4, :], in_=o_s[:, 2:4, :])
                else:
                    nc.sync.dma_start(out=hbm, in_=o_s[:, :, :])

            TOT = bpc * NQT * NPAIR
            atts = [None] * TOT
            avs = {}    # pass index -> (av_t list, o_s)

            for g in range(TOT + PD):
                # just-in-time loads: at mid-pass (g%8==4) issue the q-tile
                # loads for pass p+2 (~1.5-pass lead on SP's serial queue);
                # batch-1 kt/va ride earlier boundaries.
                if g < TOT and (g % NPAIR == 4 or g == 0):
                    p2 = 1 if g == 0 else g // NPAIR + 2
                    if p2 < bpc * NQT:
                        b2, qt2 = divmod(p2, NQT)
                        _ldq(b2, qt2, SP)
                        _ldm16(b2, qt2, SP)
                        _ldm8(b2, qt2, SP)
                if g == 8:
                    _ldk(1, 0, SP)
                elif g == 11:
                    _ldk(1, 1, SP)
                elif g == 14:
                    _ldk(1, 2, SP)
                elif g == 17:
                    _ldk(1, 3, SP)
                elif g == 19:
                    _ldk(1, 4, SP)
                elif g == 21:
                    _ldva(1, 0, 8, POOL)
                    _ldva(1, 8, KB, POOL)
                # AV block of step g-PD first: if QK(g) must wait on its
                # s_ps buffer (exp of g-SPSUM_BUFS still draining), the PE
                # does these matmuls instead of idling.
                gp = g - PD
                if gp >= 0:
                    bp = gp // (NQT * NPAIR)
                    tp = gp % NPAIR
                    pidx = gp // NPAIR
                    if tp == 0:
                        avs[pidx] = (
                            [
                                avpsum.tile(
                                    [P, 2, D + 1], f32, name="av", tag="av"
                                )
                                for _ in range(2)
                            ],
                            outp.tile([P, 4, D], bf16, name="o_s", tag="o_s"),
                        )
                    av_t, o_s = avs[pidx]
                    # tile-major order: av_t[0] closes (stop) a few matmuls
                    # before av_t[1], so the final norm starts earlier
                    for tile in range(2):
                        for h in range(2):
                            kb = 2 * tp + h
                            for sl in (2 * tile, 2 * tile + 1):
                                nc.tensor.matmul(
                                    av_t[tile][:, sl % 2, :],
                                    lhsT=atts[gp][:, h, sl * P:(sl + 1) * P],
                                    rhs=va_s[bp][:, kb, :],
                                    start=(kb == 0 and sl % 2 == 0),
                                    stop=(kb == KB - 1 and sl % 2 == 1),
                                )
                    if tp == NPAIR - 1:
                        qtp = (gp // NPAIR) % NQT
                        emit_norm(av_t, o_s, bp, qtp,
                                  split=(pidx == bpc * NQT - 1))
                        del avs[pidx]

                if g < TOT:
                    b = g // (NQT * NPAIR)
                    qt = (g // NPAIR) % NQT
                    t = g % NPAIR
                    s_ps = spsum.tile([P, 2, QW], f32, name="s_ps", tag="s")
                    for h in range(2):
                        kb = 2 * t + h
                        kc = next(
                            i for i in range(len(KCW))
                            if KCO[i] <= kb * P < KCO[i] + KCW[i]
                        )
                        ko = kb * P - KCO[kc]
                        nc.tensor.matmul(
                            s_ps[:, h, :],
                            lhsT=kt_ch[b][kc][:, ko:ko + P],
                            rhs=qt_ch[b][qt][:],
                            start=True,
                            stop=True,
                        )
                    path = PATHS[t]
                    att = attnp.tile([P, 2, QW], bf16, name="att", tag="att")
                    if path == "SCHR":
                        pos = M8_POS[t]
                        att1 = schrip.tile(
                            [P, 2, QW], bf16, name="att1", tag="schri"
                        )
                        att2 = schrip.tile(
                            [P, 2, QW], bf16, name="att2", tag="schri"
                        )
                        # PSUM-reading ops stay on DVE (GPSIMD may not
                        # access PSUM); SBUF-only tail split DVE/Pool
                        nc.vector.scalar_tensor_tensor(
                            out=att1[:].bitcast(i16),
                            in0=m8_t[b][qt][:, 2 * pos:2 * pos + 2, :],
                            scalar=mscale,
                            in1=s_ps[:],
                            op0=mybir.AluOpType.mult,
                            op1=mybir.AluOpType.add,
                        )
                        nc.vector.tensor_scalar(
                            out=att2[:].bitcast(i16),
                            in0=att1[:].bitcast(i16),
                            scalar1=64,
                            scalar2=None,
                            op0=mybir.AluOpType.add,
                        )
                        nc.vector.tensor_scalar(
                            out=att2[:],
                            in0=att2[:],
                            scalar1=invs2,
                            scalar2=None,
                            op0=mybir.AluOpType.mult,
                        )
                        nc.gpsimd.tensor_add(att[:], att2[:], att1[:])
                    else:
                        att_e = attnep.tile(
                            [P, 2, QW], bf16, name="att_e", tag="atte"
                        )
                        nc.scalar.activation(
                            att_e[:],
                            s_ps[:],
                            mybir.ActivationFunctionType.Exp,
                            scale=escale,
                        )
                        if path == "E16":
                            pos = M16_POS[t]
                            nc.vector.tensor_mul(
                                att[:], att_e[:],
                                m16_t[b][qt][:, 2 * pos:2 * pos + 2, :],
                            )
                        else:  # EU8P
                            pos = M8_POS[t]
                            nc.gpsimd.tensor_mul(
                                att[:], att_e[:],
                                m8_t[b][qt][:, 2 * pos:2 * pos + 2, :],
                            )
                    atts[g] = att
    nc.compile()
    return nc


def _prep_inputs(q, k, v, mask):
    bf = ml_dtypes.bfloat16
    ones = np.ones((B, S, 1), dtype=np.float32)
    va_full = (
        np.concatenate([np.asarray(v, np.float32), ones], axis=2)
        .reshape(B, S // P, P, D + 1)
        .transpose(0, 2, 1, 3)
        .reshape(B, P, (S // P) * (D + 1))
        .astype(bf)
    )
    ascale = np.float32(P / np.log(2.0) / np.sqrt(D))
    qT_full = np.ascontiguousarray(
        np.asarray(q, np.float32).transpose(0, 2, 1) * ascale
    ).astype(bf)
    kT_full = np.ascontiguousarray(
        np.asarray(k, np.float32).transpose(0, 2, 1)
    ).astype(bf)
    mT = np.asarray(mask, bool).transpose(0, 2, 1)  # [b, k, q]
    m16_full = np.empty((B, len(M16_PAIRS) * 2 * P, S), dtype=np.int16)
    for i, pr in enumerate(M16_PAIRS):
        blk = mT[:, 2 * pr * P:(2 * pr + 2) * P, :]
        r0, r1 = 2 * i * P, (2 * i + 2) * P
        m16_full[:, r0:r1, :] = np.where(blk, np.int16(0), np.int16(1))
    m8_full = np.empty((B, len(M8_PAIRS) * 2 * P, S), dtype=np.uint8)
    for i, pr in enumerate(M8_PAIRS):
        blk = mT[:, 2 * pr * P:(2 * pr + 2) * P, :]
        r0, r1 = 2 * i * P, (2 * i + 2) * P
        if PATHS[pr] == "SCHR":
            m8_full[:, r0:r1, :] = np.where(
                blk, np.uint8(SCHR_M_DEAD), np.uint8(SCHR_M_LIVE)
            )
        else:
            m8_full[:, r0:r1, :] = (~blk).astype(np.uint8)
    return qT_full, kT_full, va_full, m16_full, m8_full


def kernel(q, k, v, mask, _trace=False, _trace_kwargs=None):
    global _NC, LAST_RESULT
    from concourse.bass_utils import run_bass_kernel_spmd

    if _NC is None:
        _NC = _build_nc()

    qT_full, kT_full, va_full, m16_full, m8_full = _prep_inputs(q, k, v, mask)

    in_maps = []
    for c in range(N_CORES):
        lo, hi = c * BPC, (c + 1) * BPC
        in_maps.append(
            {
                "qT": qT_full[lo:hi],
                "kT": kT_full[lo:hi],
                "va": va_full[lo:hi],
                "m16": m16_full[lo:hi],
                "m8": m8_full[lo:hi],
            }
        )

    kw = {}
    if _trace:
        kw["trace"] = True
        if _trace_kwargs:
            kw.update(_trace_kwargs)
    LAST_RESULT = run_bass_kernel_spmd(_NC, in_maps, list(range(N_CORES)), **kw)
    res = LAST_RESULT.results
    outp = np.concatenate(
        [np.asarray(res[c]["out"]) for c in range(N_CORES)], axis=0
    ).astype(np.float32)
    # [b, qt, p, sl*D+d] -> [b, qt*512 + sl*128 + p, d]
    out = (
        outp.reshape(B, NQT, P, 4, D)
        .transpose(0, 1, 3, 2, 4)
        .reshape(B, S, D)
    )
    return np.ascontiguousarray(out)
